# revision 22
# baseline (speedup 1.0000x reference)
"""AltConv via Winograd F(8,4) fp16 on 8 TRN2 NeuronCores.

out[s] = sum_{i=0..3} K_i x[s-i].  8 outputs per block from 11
Winograd-channel matmuls (vs 32 direct): points
{4, +-1, +-2, +-3/4, +-1/2, 0, inf}.

  w_l(u) = x[8u-3+l], l=0..10
  x~_j = cs_j * sum_l BT[j,l] w_l    (host, f64 -> fp16)
  K~_j = ds_j * sum_i G[j,i] K_{3-i}   (host, f64 -> fp16)
  P_j  = x~_j @ K~_j                 (device TensorE, f32 PSUM, staged
                                      fp16 by ScalarE and DMA'd out)
  out[8u+t] = sum_j (p_j^t/(cs_j ds_j)) P_j   (host, f32 einsum)

The device does only the matmul core (all of the conv's O(S D F) FLOPs);
the O(S F) input/output transforms run on host.  Per-channel pow2 scales
cs/ds keep every fp16 tensor in normal range (sim rel err 8.1e-3, gate
2e-2, immune to subnormal flush).

Sharding: data-parallel over (batch, seq-half) -> 8 shards of 4096
tokens = 512 blocks; U=512 makes each PSUM tile exactly one bank, one
chunk, no tail.  x~ SBUF-resident (90 KB/partition); kernel F-block
slices stream through a 3-deep pool.  Per fb: 88 matmuls of 512 cols
back-to-back; the only non-PE device work is 11 ScalarE PSUM->fp16
copies and 11 output DMAs per fb, so TensorE runs unthrottled.
"""

import math
import numpy as np

B, S, D, F, R = 4, 8192, 1024, 1024, 4
N_CORES = 8
T = S // 2            # tokens per core
M = 8                 # outputs per Winograd block
POINTS = [4.0, 1.0, -1.0, 2.0, -2.0, 0.75, -0.75, 0.5, -0.5, 0.0]  # + inf
NJ = len(POINTS) + 1  # 11 channels
KD = D // 128
FB = F // 128
U = T // M            # 512 blocks, exactly
_CACHE = {}


def _transforms():
    n = NJ
    V = np.zeros((n, n))
    for j, p in enumerate(POINTS):
        V[j] = [p ** e for e in range(n)]
    V[-1, -1] = 1.0
    BT = np.linalg.inv(V).T
    G = np.zeros((n, R))
    for j, p in enumerate(POINTS):
        G[j] = [p ** e for e in range(R)]
    G[-1, R - 1] = 1.0
    # per-channel power-of-2 scales from the input distribution
    # (x ~ N(0,1), k ~ N(0, 1/(R*D)))
    sigk = 1.0 / math.sqrt(R) / math.sqrt(D)
    cs, ds = np.ones(n), np.ones(n)
    for j in range(n):
        cs[j] = 2.0 ** round(math.log2(1.0 / np.linalg.norm(BT[j])))
        ds[j] = 2.0 ** round(math.log2(1.0 / (np.linalg.norm(G[j]) * sigk)))
    for j, p in enumerate(POINTS):
        for j2, p2 in enumerate(POINTS):
            if p2 == -p and p != 0 and j2 > j:
                cs[j2], ds[j2] = cs[j], ds[j]
    return BT, G, cs, ds


def _build():
    if "nc" in _CACHE:
        return _CACHE["nc"]
    import concourse.tile as tile
    from concourse import bacc, mybir

    nc = bacc.Bacc("TRN2", target_bir_lowering=False, debug=False,
                   num_devices=N_CORES)
    f16 = mybir.dt.float16
    f32 = mybir.dt.float32

    xt_d = nc.dram_tensor("xt", [128, NJ, KD, U], f16, kind="ExternalInput")
    kt_d = nc.dram_tensor("kt", [FB, 128, NJ, KD, 128], f16,
                          kind="ExternalInput")
    out_d = nc.dram_tensor("outT", [FB, 128, NJ, U], f16,
                           kind="ExternalOutput")

    with tile.TileContext(nc) as tc:
        with (
            tc.tile_pool(name="kpool", bufs=3) as kpool,
            tc.tile_pool(name="xpool", bufs=1) as xpool,
            tc.tile_pool(name="psum", bufs=1, space="PSUM") as ppool,
            tc.tile_pool(name="sd", bufs=1) as sdpool,
        ):
            xt = xpool.tile([128, NJ, KD, U], f16, name="xt", tag="xt")
            warm = sdpool.tile([128, 512], f16, name="warm", tag="warm")
            nc.vector.memset(warm[:, :], 0.0)
            Pw = ppool.tile([128, 512], f32, tag="Pw", name="Pwarm", bufs=1)
            for _ in range(10):
                nc.tensor.matmul(Pw, warm[:, :128], warm, start=True,
                                 stop=True)
            # ---- front: fbs 0-2 interleaved j-wise -------------------
            # The fill phase moves xt (11.5 MB) + kt0-2 (8.7 MB) at ring
            # bandwidth (~56 us).  Interleaving three fbs j-wise gives the
            # PE ~56 us of real work to overlap instead of idling on fb0
            # alone.  Per-queue delivery stays in consumption order:
            #   sync:   xt[j], kt2[j] alternating
            #   scalar: kt0[j]        gpsimd: kt1[j]
            # Output DMAs of fb0/fb1 are deferred past the fill phase.
            NFRONT = 3
            sds = {}
            kts = {}
            for fb in range(NFRONT):
                kts[fb] = kpool.tile([128, NJ, KD, 128], f16,
                                     name=f"kt{fb}", tag="kt")
                sds[fb] = sdpool.tile([128, NJ, U], f16, name=f"sd{fb}",
                                      tag="sd", bufs=4)
            for j in range(NJ):
                if j < 3:
                    # quarter-grain early xt so the PE never waits long
                    for kh in range(4):
                        ks = slice(kh * (KD // 4), (kh + 1) * (KD // 4))
                        nc.scalar.dma_start(kts[0][:, j, ks],
                                            kt_d[0, :, j, ks])
                        nc.sync.dma_start(xt[:, j, ks], xt_d[:, j, ks])
                else:
                    nc.scalar.dma_start(kts[0][:, j], kt_d[0, :, j])
                    nc.sync.dma_start(xt[:, j], xt_d[:, j])
                nc.gpsimd.dma_start(kts[1][:, j], kt_d[1, :, j])
                nc.sync.dma_start(kts[2][:, j], kt_d[2, :, j])
                for fb in range(NFRONT):
                    P = ppool.tile([128, U], f32, tag="pp",
                                   name=f"P{fb}_{j}", bufs=4)
                    for kd in range(KD):
                        nc.tensor.matmul(
                            P, kts[fb][:, j, kd, :], xt[:, j, kd, :],
                            start=(kd == 0), stop=(kd == KD - 1),
                        )
                    nc.scalar.copy(sds[fb][:, j, :], P)
                # 3 dummy matmuls per j-group keep the PE duty cycle high
                # enough for the HAM clock gate during the fill
                for _ in range(3):
                    nc.tensor.matmul(Pw, warm[:, :128], warm, start=True,
                                     stop=True)
            # fb2's outputs can go out right away (fill phase is over by
            # its last channel); fb0/fb1 flush during fb3/fb4
            nc.gpsimd.dma_start(out_d[2], sds[2])
            deferred = [0, 1]
            # ---- steady state: fbs 3-7, j-wise kt loads, one-ahead ----
            kt3 = kpool.tile([128, NJ, KD, 128], f16, name="kt3", tag="kt")
            for j in range(NJ):
                nc.scalar.dma_start(kt3[:, j], kt_d[3, :, j])
            kts[3] = kt3
            for fb in range(NFRONT, FB):
                if fb + 1 < FB:
                    ktn = kpool.tile([128, NJ, KD, 128], f16,
                                     name=f"kt{fb + 1}", tag="kt")
                    keng = nc.scalar if (fb + 1) % 2 else nc.gpsimd
                    for j in range(NJ):
                        keng.dma_start(ktn[:, j], kt_d[fb + 1, :, j])
                    kts[fb + 1] = ktn
                # flush deferred fb0/fb1 outputs in the fb6/fb7 windows,
                # after the kt stream has finished (real DMA slack there)
                if fb >= FB - 2 and deferred:
                    dfb = deferred.pop(0)
                    eng2 = nc.gpsimd if fb % 2 else nc.scalar
                    eng2.dma_start(out_d[dfb], sds.pop(dfb))
                kt = kts[fb]
                sd = sdpool.tile([128, NJ, U], f16, name=f"sd{fb}",
                                 tag="sd", bufs=4)
                for j in range(NJ):
                    P = ppool.tile([128, U], f32, tag="pp",
                                   name=f"P{fb}_{j}", bufs=4)
                    for kd in range(KD):
                        nc.tensor.matmul(
                            P, kt[:, j, kd, :], xt[:, j, kd, :],
                            start=(kd == 0), stop=(kd == KD - 1),
                        )
                    nc.scalar.copy(sd[:, j, :], P)
                    if fb == FB - 1:
                        # drain the last fb's outputs as they appear so
                        # the tail only waits on the final channel
                        if j == 4:
                            nc.gpsimd.dma_start(out_d[fb, :, :5, :],
                                                sd[:, :5, :])
                        elif j == NJ - 2:
                            nc.gpsimd.dma_start(out_d[fb, :, 5:NJ - 1, :],
                                                sd[:, 5:NJ - 1, :])
                if fb == FB - 1:
                    nc.scalar.dma_start(out_d[fb, :, NJ - 1, :],
                                        sd[:, NJ - 1, :])
                else:
                    eng = nc.scalar if fb % 2 else nc.gpsimd
                    eng.dma_start(out_d[fb], sd)

    nc.compile()
    _CACHE["nc"] = nc
    return nc


def _prep_inputs(x, kernels):
    f16 = np.float16
    BT, G, cs, ds = _transforms()
    Kt = np.einsum("ji,idf->jdf", G, kernels[::-1].astype(np.float64))
    Kt *= ds[:, None, None]
    kt_f16 = np.ascontiguousarray(
        Kt.reshape(NJ, KD, 128, FB, 128).transpose(3, 2, 0, 1, 4).astype(f16))
    in_maps = []
    for c in range(N_CORES):
        b, h = divmod(c, 2)
        # w_l(u) = x[b, h*T + 8u - 3 + l]; rows outside [0, S) are zero
        need = M * (U - 1) + NJ           # 4099 window rows
        xp = np.zeros((need, D), dtype=np.float64)
        s0 = h * T - (R - 1)
        lo, hi = max(s0, 0), min(s0 + need, S)
        xp[lo - s0: hi - s0] = x[b, lo: hi]
        idx = M * np.arange(U)
        Wn = np.stack([xp[idx + l] for l in range(NJ)])      # [11, U, D]
        Xt = np.einsum("jl,lud->jud", BT, Wn)                # [11, U, D]
        Xt *= cs[:, None, None]
        Xr = Xt.reshape(NJ, U, KD, 128).transpose(3, 0, 2, 1)  # [dp,j,kd,u]
        in_maps.append({"kt": kt_f16,
                        "xt": np.ascontiguousarray(Xr.astype(f16))})
    return in_maps


def kernel(x, kernels, biases, trace=False):
    from concourse.bass_utils import run_bass_kernel_spmd

    x = np.asarray(x, dtype=np.float32)
    kernels = np.asarray(kernels, dtype=np.float32)
    biases = np.asarray(biases, dtype=np.float32)
    nc = _build()
    in_maps = _prep_inputs(x, kernels)
    res = run_bass_kernel_spmd(nc, in_maps, core_ids=list(range(N_CORES)),
                               trace=trace)
    _, _, cs, ds = _transforms()
    A = np.zeros((M, NJ), dtype=np.float32)
    for j, p in enumerate(POINTS):
        A[:, j] = [p ** t / (cs[j] * ds[j]) for t in range(M)]
    A[:, -1] = 0.0
    A[M - 1, -1] = 1.0 / (cs[-1] * ds[-1])
    out = np.empty((B, S, F), dtype=np.float32)
    for c in range(N_CORES):
        b, h = divmod(c, 2)
        o = np.asarray(res.results[c]["outT"]).astype(np.float32)
        # o: [FB, 128, NJ, U]; token h*T + 8u + t, feature fb*128 + fp
        rows = np.einsum("tj,apju->utap", A, o)      # [U, M, FB, 128]
        out[b, h * T:(h + 1) * T, :] = rows.reshape(T, F)
    bias_total = biases.astype(np.float32).sum(axis=0)
    if np.any(bias_total):
        out += bias_total
    if trace:
        kernel.last_exec_time_ns = res.exec_time_ns
    return out


# revision 24
# speedup vs baseline: 1.0659x; 1.0659x over previous
"""AltConv via Winograd F(8,4) fp16 on 8 TRN2 NeuronCores.

out[s] = sum_{i=0..3} K_i x[s-i].  8 outputs per block from 11
Winograd-channel matmuls (vs 32 direct): points
{4, +-1, +-2, +-3/4, +-1/2, 0, inf}.

  w_l(u) = x[8u-3+l], l=0..10
  x~_j = cs_j * sum_l BT[j,l] w_l    (host, f64 -> fp16)
  K~_j = ds_j * sum_i G[j,i] K_{3-i}   (host, f64 -> fp16)
  P_j  = x~_j @ K~_j                 (device TensorE, f32 PSUM, staged
                                      fp16 by ScalarE and DMA'd out)
  out[8u+t] = sum_j (p_j^t/(cs_j ds_j)) P_j   (host, f32 einsum)

The device does only the matmul core (all of the conv's O(S D F) FLOPs);
the O(S F) input/output transforms run on host.  Per-channel pow2 scales
cs/ds keep every fp16 tensor in normal range (sim rel err 8.1e-3, gate
2e-2, immune to subnormal flush).

Sharding: data-parallel over (batch, seq-half) -> 8 shards of 4096
tokens = 512 blocks; U=512 makes each PSUM tile exactly one bank, one
chunk, no tail.  x~ SBUF-resident (88 KB/partition).

Schedule: the fill phase (xt 11.5 MB + kt0-2 8.7 MB at ring bandwidth,
~56 us) interleaves fbs 0-2 j-wise so the PE has ~56 us of real work to
overlap.  Kernel slices stream through two 3-deep half-tile pools
(channels 0-5 / 6-10); a half frees mid-fb, so following fbs' kernel
halves trickle in just-in-time without bandwidth spikes.  Early output
DMAs are deferred out of the fill phase.
"""

import math
import numpy as np

B, S, D, F, R = 4, 8192, 1024, 1024, 4
N_CORES = 8
T = S // 2            # tokens per core
M = 8                 # outputs per Winograd block
POINTS = [4.0, 1.0, -1.0, 2.0, -2.0, 0.75, -0.75, 0.5, -0.5, 0.0]  # + inf
NJ = len(POINTS) + 1  # 11 channels
NA = 6                # channels in the first kernel half-tile
KD = D // 128
FB = F // 128
U = T // M            # 512 blocks, exactly
_CACHE = {}


def _transforms():
    n = NJ
    V = np.zeros((n, n))
    for j, p in enumerate(POINTS):
        V[j] = [p ** e for e in range(n)]
    V[-1, -1] = 1.0
    BT = np.linalg.inv(V).T
    G = np.zeros((n, R))
    for j, p in enumerate(POINTS):
        G[j] = [p ** e for e in range(R)]
    G[-1, R - 1] = 1.0
    # per-channel power-of-2 scales from the input distribution
    # (x ~ N(0,1), k ~ N(0, 1/(R*D)))
    sigk = 1.0 / math.sqrt(R) / math.sqrt(D)
    cs, ds = np.ones(n), np.ones(n)
    for j in range(n):
        cs[j] = 2.0 ** round(math.log2(1.0 / np.linalg.norm(BT[j])))
        ds[j] = 2.0 ** round(math.log2(1.0 / (np.linalg.norm(G[j]) * sigk)))
    for j, p in enumerate(POINTS):
        for j2, p2 in enumerate(POINTS):
            if p2 == -p and p != 0 and j2 > j:
                cs[j2], ds[j2] = cs[j], ds[j]
    return BT, G, cs, ds


def _build():
    if "nc" in _CACHE:
        return _CACHE["nc"]
    import concourse.tile as tile
    from concourse import bacc, mybir

    nc = bacc.Bacc("TRN2", target_bir_lowering=False, debug=False,
                   num_devices=N_CORES)
    f16 = mybir.dt.float16
    f32 = mybir.dt.float32

    xt_d = nc.dram_tensor("xt", [128, NJ, KD, U], f16, kind="ExternalInput")
    kt_d = nc.dram_tensor("kt", [FB, 128, NJ, KD, 128], f16,
                          kind="ExternalInput")
    out_d = nc.dram_tensor("outT", [FB, 128, NJ, U], f16,
                           kind="ExternalOutput")

    with tile.TileContext(nc) as tc:
        with (
            tc.tile_pool(name="kpa", bufs=3) as kpa,
            tc.tile_pool(name="kpb", bufs=3) as kpb,
            tc.tile_pool(name="xpool", bufs=1) as xpool,
            tc.tile_pool(name="psum", bufs=1, space="PSUM") as ppool,
            tc.tile_pool(name="sd", bufs=1) as sdpool,
        ):
            xt = xpool.tile([128, NJ, KD, U], f16, name="xt", tag="xt")
            warm = sdpool.tile([128, 512], f16, name="warm", tag="warm")
            nc.vector.memset(warm[:, :], 0.0)
            Pw = ppool.tile([128, 512], f32, tag="Pw", name="Pwarm", bufs=1)
            for _ in range(10):
                nc.tensor.matmul(Pw, warm[:, :128], warm, start=True,
                                 stop=True)

            ktsA, ktsB, sds = {}, {}, {}

            def kt_alloc(fb):
                ktsA[fb] = kpa.tile([128, NA, KD, 128], f16,
                                    name=f"ktA{fb}", tag="ktA")
                ktsB[fb] = kpb.tile([128, NJ - NA, KD, 128], f16,
                                    name=f"ktB{fb}", tag="ktB")

            def kt_slice(fb, j):
                if j < NA:
                    return ktsA[fb][:, j]
                return ktsB[fb][:, j - NA]

            def kt_load_j(eng, fb, j, quarters=False):
                if quarters:
                    for kh in range(4):
                        ks = slice(kh * (KD // 4), (kh + 1) * (KD // 4))
                        eng.dma_start(kt_slice(fb, j)[:, ks],
                                      kt_d[fb, :, j, ks])
                else:
                    eng.dma_start(kt_slice(fb, j), kt_d[fb, :, j])

            # ---- front: fbs 0-2 interleaved j-wise -------------------
            # Per-queue delivery stays in consumption order:
            #   sync:   xt[j], kt2[j] alternating
            #   scalar: kt0[j]        gpsimd: kt1[j]
            NFRONT = 3
            for fb in range(NFRONT):
                kt_alloc(fb)
                sds[fb] = sdpool.tile([128, NJ, U], f16, name=f"sd{fb}",
                                      tag="sd", bufs=4)
            for j in range(NJ):
                if j == 0:
                    kt_load_j(nc.scalar, 0, 0, quarters=True)
                    for kh in range(4):
                        ks = slice(kh * (KD // 4), (kh + 1) * (KD // 4))
                        nc.sync.dma_start(xt[:, 0, ks], xt_d[:, 0, ks])
                else:
                    kt_load_j(nc.scalar, 0, j)
                    nc.sync.dma_start(xt[:, j], xt_d[:, j])
                kt_load_j(nc.gpsimd, 1, j)
                kt_load_j(nc.sync, 2, j)
                for fb in range(NFRONT):
                    P = ppool.tile([128, U], f32, tag="pp",
                                   name=f"P{fb}_{j}", bufs=4)
                    for kd in range(KD):
                        nc.tensor.matmul(
                            P, kt_slice(fb, j)[:, kd, :], xt[:, j, kd, :],
                            start=(kd == 0), stop=(kd == KD - 1),
                        )
                    nc.scalar.copy(sds[fb][:, j, :], P)
                # 2 dummy matmuls per j-group keep the PE duty cycle high
                # enough for the HAM clock gate during the fill
                for _ in range(2):
                    nc.tensor.matmul(Pw, warm[:, :128], warm, start=True,
                                     stop=True)
            # fb2's outputs can go out right away (fill phase is over by
            # its last channel); fb0/fb1 flush during fb3/fb4
            nc.gpsimd.dma_start(out_d[2], sds[2])
            deferred = [0, 1]
            # ---- steady state ----------------------------------------
            # kt halves stream just-in-time: a half-tile buffer frees as
            # soon as its 6 (or 5) channels are consumed mid-fb, so the
            # following fbs' halves trickle in without bandwidth spikes.
            kt_alloc(3)
            kt_alloc(4)
            for j in range(NJ):
                kt_load_j(nc.scalar, 3, j)
            nc.gpsimd.dma_start(ktsA[4][:, :], kt_d[4, :, :NA])
            nc.gpsimd.dma_start(ktsB[4][:, :], kt_d[4, :, NA:])
            for fb in range(NFRONT, FB):
                if fb + 2 < FB:
                    kt_alloc(fb + 2)
                    keng = nc.scalar if fb % 2 else nc.gpsimd
                    keng.dma_start(ktsA[fb + 2][:, :], kt_d[fb + 2, :, :NA])
                    keng.dma_start(ktsB[fb + 2][:, :], kt_d[fb + 2, :, NA:])
                if deferred:
                    dfb = deferred.pop(0)
                    eng2 = nc.gpsimd if fb % 2 else nc.scalar
                    eng2.dma_start(out_d[dfb], sds.pop(dfb))
                sd = sdpool.tile([128, NJ, U], f16, name=f"sd{fb}",
                                 tag="sd", bufs=4)
                for j in range(NJ):
                    P = ppool.tile([128, U], f32, tag="pp",
                                   name=f"P{fb}_{j}", bufs=4)
                    for kd in range(KD):
                        nc.tensor.matmul(
                            P, kt_slice(fb, j)[:, kd, :], xt[:, j, kd, :],
                            start=(kd == 0), stop=(kd == KD - 1),
                        )
                    nc.scalar.copy(sd[:, j, :], P)
                    if fb == FB - 1:
                        # drain the last fb's outputs as they appear so
                        # the tail only waits on the final channel
                        if j == 4:
                            nc.gpsimd.dma_start(out_d[fb, :, :5, :],
                                                sd[:, :5, :])
                        elif j == NJ - 2:
                            nc.gpsimd.dma_start(out_d[fb, :, 5:NJ - 1, :],
                                                sd[:, 5:NJ - 1, :])
                if fb == FB - 1:
                    nc.scalar.dma_start(out_d[fb, :, NJ - 1, :],
                                        sd[:, NJ - 1, :])
                else:
                    eng = nc.scalar if fb % 2 else nc.gpsimd
                    eng.dma_start(out_d[fb], sd)

    nc.compile()
    _CACHE["nc"] = nc
    return nc


def _prep_inputs(x, kernels):
    f16 = np.float16
    BT, G, cs, ds = _transforms()
    Kt = np.einsum("ji,idf->jdf", G, kernels[::-1].astype(np.float64))
    Kt *= ds[:, None, None]
    kt_f16 = np.ascontiguousarray(
        Kt.reshape(NJ, KD, 128, FB, 128).transpose(3, 2, 0, 1, 4).astype(f16))
    in_maps = []
    for c in range(N_CORES):
        b, h = divmod(c, 2)
        # w_l(u) = x[b, h*T + 8u - 3 + l]; rows outside [0, S) are zero
        need = M * (U - 1) + NJ           # 4099 window rows
        xp = np.zeros((need, D), dtype=np.float64)
        s0 = h * T - (R - 1)
        lo, hi = max(s0, 0), min(s0 + need, S)
        xp[lo - s0: hi - s0] = x[b, lo: hi]
        idx = M * np.arange(U)
        Wn = np.stack([xp[idx + l] for l in range(NJ)])      # [11, U, D]
        Xt = np.einsum("jl,lud->jud", BT, Wn)                # [11, U, D]
        Xt *= cs[:, None, None]
        Xr = Xt.reshape(NJ, U, KD, 128).transpose(3, 0, 2, 1)  # [dp,j,kd,u]
        in_maps.append({"kt": kt_f16,
                        "xt": np.ascontiguousarray(Xr.astype(f16))})
    return in_maps


def kernel(x, kernels, biases, trace=False):
    from concourse.bass_utils import run_bass_kernel_spmd

    x = np.asarray(x, dtype=np.float32)
    kernels = np.asarray(kernels, dtype=np.float32)
    biases = np.asarray(biases, dtype=np.float32)
    nc = _build()
    in_maps = _prep_inputs(x, kernels)
    res = run_bass_kernel_spmd(nc, in_maps, core_ids=list(range(N_CORES)),
                               trace=trace)
    _, _, cs, ds = _transforms()
    A = np.zeros((M, NJ), dtype=np.float32)
    for j, p in enumerate(POINTS):
        A[:, j] = [p ** t / (cs[j] * ds[j]) for t in range(M)]
    A[:, -1] = 0.0
    A[M - 1, -1] = 1.0 / (cs[-1] * ds[-1])
    out = np.empty((B, S, F), dtype=np.float32)
    for c in range(N_CORES):
        b, h = divmod(c, 2)
        o = np.asarray(res.results[c]["outT"]).astype(np.float32)
        # o: [FB, 128, NJ, U]; token h*T + 8u + t, feature fb*128 + fp
        rows = np.einsum("tj,apju->utap", A, o)      # [U, M, FB, 128]
        out[b, h * T:(h + 1) * T, :] = rows.reshape(T, F)
    bias_total = biases.astype(np.float32).sum(axis=0)
    if np.any(bias_total):
        out += bias_total
    if trace:
        kernel.last_exec_time_ns = res.exec_time_ns
    return out
